# revision 32
# baseline (speedup 1.0000x reference)
"""DiT block kernel for Trainium2, SPMD data-parallel over batch across 8 NeuronCores.

Per-core computation (one batch element, N=1024 tokens, D=1024):
  adaLN1 -> qkv -> attention(16 heads, hd=64) -> proj + residual
  adaLN2 -> fc1 -> gelu(exact/erf) -> fc2 + residual

Layout strategy (v2):
  - residual stream x kept token-major (tm) [tok_p, feat] in SBUF
  - LN split in two: tm pass applies (x-mu)*rstd (per-token scalars are
    per-partition in tm); the adaLN (1+scale)/shift are per-FEATURE, so they
    are applied feature-major (fm) as per-partition scalars fused into the
    PSUM->SBUF copy right after the PE transpose. No broadcast matmuls.
  - adaLN scale/shift rows are transposed to fm columns via a tiny
    SBUF->DRAM->SBUF bounce (DRAM is linear, so the relayout is free).
  - attention S^T computed feature-major per head with 64-partition
    contraction (no zero padding); softmax without max subtraction (exact
    same math as the reference; fp32 psum).
  - V carries the v-bias folded in (ones-matmul into the V psum) plus a
    ones column, so AV yields both bias-corrected context and softmax
    denominators; tail = reciprocal + PE broadcast + one tensor_tensor.
  - attention units ordered qg-major so all heads of the first 512 tokens
    finish first and proj+LN2 for those tokens overlaps the second half
    of attention.
  - all weight DMAs are single batched descriptors (>=512B lines) on the
    SP queue; x/out/constants/small shuffles ride the Pool (swdge) queue.
  - fc2 in 512-wide psum passes.
"""

import sys

if "/opt/trn_rl_repo" not in sys.path:
    sys.path.insert(0, "/opt/trn_rl_repo")

from contextlib import ExitStack

import ml_dtypes
import numpy as np

import concourse.bacc as bacc
import concourse.bass as bass
import concourse.mybir as mybir
import concourse.tile as tile
from concourse.bass import ds, ts
from concourse.masks import make_identity

FP32 = mybir.dt.float32
BF16 = mybir.dt.bfloat16
AF = mybir.ActivationFunctionType
ALU = mybir.AluOpType

B, N, D = 8, 1024, 1024
H, HD, DFF = 16, 64, 4096
P = 128
NT = N // P   # 8 token tiles
KT = D // P   # 8 feature k-tiles
EPS = 1e-6
# "erf": exact gelu via Erf activation (not implemented in CoreSim, HW ok)
# "tanh": tanh-approx gelu from Square+Tanh (CoreSim-compatible fallback)
GELU_MODE = "erf"

BF16_NP = ml_dtypes.bfloat16


def build():
    """Build the single-core program (same program on all 8 cores)."""
    nc = bacc.Bacc(None, target_bir_lowering=False, debug=False)
    names = {}

    with tile.TileContext(nc) as tc:
        with ExitStack() as root:
            dram = root.enter_context(tc.tile_pool(name="dram", bufs=1, space="DRAM"))

            def din(nm, shape, dt=BF16):
                t = dram.tile(shape, dt, kind="ExternalInput", name=nm)
                names[nm] = t.name
                return t

            x_d = din("x", [N, D])  # bf16 (residual re-materialized in fp32)
            wqk_d = din("wqk", [16, P, D])          # per oft: [feat_p, kt*128]
            wv_d = din("wv", [2, P, 8 * 512])       # per og: [feat_p, kt*512]
            wproj_d = din("wproj", [2, P, 8 * 512])
            wada1_d = din("wada1", [4, P, 8 * 512])
            wada2_d = din("wada2", [4, P, 8 * 512])
            wfc1_d = din("wfc1", [32, P, D])        # per oft: [feat_p, kt*128]
            wfc2_d = din("wfc2", [2, P, 32 * 512])  # per oc: [ff_p, kt*512]
            cstf_d = din("cstf", [P, 112], FP32)    # bqt|bkt|bfc1t|bfc1ts|adab
            cstb_d = din("cstb", [P, 8])            # condt
            rows_d = din("rows", [1, 3 * D])        # bvt|bproj|bfc2 (bf16)
            out_d = dram.tile([N, D], FP32, kind="ExternalOutput", name="out")
            names["out"] = out_d.name
            ssb_d = dram.tile([2, 2 * D], FP32, name="ssbounce")

            # ---------------- constants / small inputs ----------------
            const = root.enter_context(tc.tile_pool(name="const", bufs=1))
            psum = root.enter_context(tc.tile_pool(name="psum", bufs=6, space="PSUM"))

            def pt(nm="ps"):
                return psum.tile([P, 512], FP32, tag="ps", name=nm, bufs=4)

            def pav(nm="pav"):
                return psum.tile([P, 512], FP32, tag="pav", name=nm, bufs=2)

            def pt_tr(nm="pstr"):
                return psum.tile([P, P], BF16, tag="pstr", name=nm, bufs=2)

            def pt_b(nm="psb"):
                # broadcast psum; shares banks with the transpose tag
                return psum.tile([P, 512], FP32, tag="pstr", name=nm, bufs=2)

            ones_bf = const.tile([1, P], BF16, name="ones_bf")
            nc.vector.memset(ones_bf[:, :], 1.0)
            ident_bf = const.tile([P, P], BF16, name="ident_bf")
            make_identity(nc, ident_bf[:, :])
            zero_col = const.tile([P, 1], FP32, name="zero_col")
            nc.vector.memset(zero_col[:, :], 0.0)
            nc.const_aps.aps[(FP32, 0.0)] = zero_col[:, :]
            eps_col = const.tile([P, 1], FP32, name="eps_col")
            nc.vector.memset(eps_col[:, :], EPS)

            cstf = const.tile([P, 112], FP32, name="cstf")
            nc.gpsimd.dma_start(out=cstf[:, :], in_=cstf_d[:, :])
            bqt_sb = cstf[:, 0:8]
            bkt_sb = cstf[:, 8:16]
            bfc1t_sb = cstf[:, 16:48]
            bfc1ts_sb = cstf[:, 48:80]
            adab_sb = cstf[:, 80:112]  # sTb1|shTb1|sTb2|shTb2 fm columns
            condt_sb = const.tile([P, 8], BF16, name="condt_sb")
            nc.gpsimd.dma_start(out=condt_sb[:, :], in_=cstb_d[:, :])
            rows_sb = const.tile([1, 3 * D], BF16, name="rows_sb")
            bvt_row = rows_sb[:, 0:D]
            bproj_sb = rows_sb[:, D : 2 * D]
            bfc2_sb = rows_sb[:, 2 * D : 3 * D]

            scr_ln = root.enter_context(tc.tile_pool(name="lnscr", bufs=2))

            # load x early (2 chunks so LN1 stats can start at the half mark)
            es_x = ExitStack()
            p_x = es_x.enter_context(tc.tile_pool(name="p_x", bufs=1))
            x_sb = p_x.tile([P, NT, D], BF16, name="x_sb")
            # x rides the scalar-engine (HWDGE) queue: ACT is idle until the
            # first softmax exp, and SP/Pool are saturated with ada weights
            xv = x_d.rearrange("(t p) d -> p t d", p=P)
            nc.scalar.dma_start(out=x_sb[:, 0:4, :], in_=xv[:, 0:4, :])

            # warm the PE clock while the first DMAs stream in
            warm_sink = const.tile([1, 8], FP32, name="warm_sink")
            wps = pt("warm")
            for _ in range(16):
                nc.tensor.matmul(
                    wps[:, 0:P], lhsT=ident_bf[:, :], rhs=ident_bf[:, :],
                    start=True, stop=True,
                )
            nc.vector.tensor_copy(out=warm_sink[:, :], in_=wps[0:1, 0:8])

            # ---------------- adaLN scale/shift columns ----------------
            # ada ai: rows (1+s)[1,D], sh[1,D] fp32 (the +1 is pre-added into
            # bada host-side), bounced via DRAM into fm columns [P, KT].
            ssT = []  # per ada: (sT [P,KT] f32, shT [P,KT] f32)

            def ada_block(ai, wada_d, wada_engs=None):
                sT = const.tile([P, KT], FP32, name=f"sT{ai}")
                shT = const.tile([P, KT], FP32, name=f"shT{ai}")
                ssv = ssb_d.rearrange("a (h k p) -> a h p k", h=2, p=P)
                if wada_engs is None:
                    wada_engs = [nc.sync, nc.sync, nc.sync, nc.gpsimd]
                with tc.tile_pool(name=f"wada{ai}", bufs=4) as wada_pool, \
                     tc.tile_pool(name=f"adas{ai}", bufs=1) as ada_scr:
                    srow = ada_scr.tile([1, 2 * D], FP32, tag="srow", name="srow")
                    # pre-issue all four weight chunks, spread across queues
                    wts = []
                    for og in range(4):
                        wt = wada_pool.tile([P, 8, 512], BF16, tag="wada", name="wadat")
                        wada_engs[og].dma_start(out=wt[:, :, :], in_=wada_d[og])
                        wts.append(wt)
                    for og in range(4):
                        ps = pt()
                        for kt in range(KT):
                            nc.tensor.matmul(
                                ps[0:1, :],
                                lhsT=condt_sb[:, kt : kt + 1],
                                rhs=wts[og][:, kt, :],
                                start=(kt == 0),
                                stop=(kt == KT - 1),
                            )
                        nc.vector.tensor_copy(
                            out=srow[:, ds(og * 512, 512)], in_=ps[0:1, :]
                        )
                        # bounce per og; the scale columns come back while the
                        # shift half is still streaming
                        nc.gpsimd.dma_start(
                            out=ssb_d[ai : ai + 1, ds(og * 512, 512)],
                            in_=srow[:, ds(og * 512, 512)],
                        )
                        if og == 1:
                            nc.gpsimd.dma_start(out=sT[:, :], in_=ssv[ai, 0])
                    nc.gpsimd.dma_start(out=shT[:, :], in_=ssv[ai, 1])
                # fold in (1+bias) columns precomputed host-side
                nc.vector.tensor_tensor(
                    out=sT[:, :], in0=sT[:, :],
                    in1=adab_sb[:, ds(16 * ai, 8)], op=ALU.add,
                )
                nc.vector.tensor_tensor(
                    out=shT[:, :], in0=shT[:, :],
                    in1=adab_sb[:, ds(16 * ai + 8, 8)], op=ALU.add,
                )
                ssT.append((sT, shT))

            ada_block(0, wada1_d, [nc.sync, nc.sync, nc.scalar, nc.gpsimd])
            nc.scalar.dma_start(out=x_sb[:, 4:8, :], in_=xv[:, 4:8, :])
            nc.scalar.dma_start(out=rows_sb[:, :], in_=rows_d[:, :])

            def ln_stats_xc(src_ap, xc_dst):
                """tm pass: xc = (x - mu) * rstd  (bf16 out)."""
                st = scr_ln.tile([P, 2, 6], FP32, tag="bnst", name="bnst")
                xr = src_ap.rearrange("p (s f) -> p s f", f=512)
                for sg in range(2):
                    nc.vector.bn_stats(out=st[:, sg, :], in_=xr[:, sg, :])
                mv = scr_ln.tile([P, 2], FP32, tag="bnmv", name="bnmv")
                nc.vector.bn_aggr(out=mv[:, :], in_=st[:, :, :])
                nc.scalar.activation(
                    out=mv[:, 1:2], in_=mv[:, 1:2], func=AF.Sqrt,
                    bias=eps_col[:, 0:1],
                )
                nc.vector.reciprocal(out=mv[:, 1:2], in_=mv[:, 1:2])
                nc.vector.tensor_scalar(
                    out=xc_dst, in0=src_ap, scalar1=mv[:, 0:1], scalar2=mv[:, 1:2],
                    op0=ALU.subtract, op1=ALU.mult,
                )

            def ln_fm(ft, tt, xc_src, sT, shT, hT):
                """fm pass: hT[:,ft,tok] = xcT * (1+s)_f + sh_f."""
                pstr = pt_tr()
                nc.tensor.transpose(pstr[:, :], xc_src, ident_bf[:, :])
                nc.vector.tensor_scalar(
                    out=hT[:, ft, ts(tt, P)], in0=pstr[:, :],
                    scalar1=sT[:, ft : ft + 1], scalar2=shT[:, ft : ft + 1],
                    op0=ALU.mult, op1=ALU.add,
                )

            # ---------------- phase B: LN1 ----------------
            es_h1 = ExitStack()
            p_h1 = es_h1.enter_context(tc.tile_pool(name="p_h1", bufs=1))
            h1T = p_h1.tile([P, KT, N], BF16, name="h1T")
            with tc.tile_pool(name="p_xc1", bufs=1) as p_xc1:
                xc1 = p_xc1.tile([P, NT, D], BF16, name="xc1")
                # per token half: stats, then an ft-outer wave batching 4
                # transposes into one [P,512] psum bank so the adaLN
                # scale/shift is a single wide tensor_scalar per (ft, half)
                for tg in range(2):
                    for tt in range(4 * tg, 4 * tg + 4):
                        ln_stats_xc(x_sb[:, tt, :], xc1[:, tt, :])
                    for ft in range(KT):
                        pw = psum.tile(
                            [P, 512], BF16, tag="pav", name="pwav", bufs=2
                        )
                        for j in range(4):
                            nc.tensor.transpose(
                                pw[:, ts(j, P)],
                                xc1[:, 4 * tg + j, ts(ft, P)],
                                ident_bf[:, :],
                            )
                        nc.vector.tensor_scalar(
                            out=h1T[:, ft, ds(tg * 512, 512)], in0=pw[:, :],
                            scalar1=ssT[0][0][:, ft : ft + 1],
                            scalar2=ssT[0][1][:, ft : ft + 1],
                            op0=ALU.mult, op1=ALU.add,
                        )

            # ada2 here: PE-cheap, and the attention-phase SBUF peak has no
            # room for its weight streaming buffers
            ada_block(1, wada2_d)

            # ------- phase C: QKV + attention + proj/LN2 (interleaved) -------
            es_qkv = ExitStack()
            p_qkv = es_qkv.enter_context(tc.tile_pool(name="p_qkv", bufs=1))
            qT = p_qkv.tile([P, KT, N], BF16, name="qT")
            kT = p_qkv.tile([P, KT, N], BF16, name="kT")
            HDP = 72  # per-head V stride: 64 values + ones col + pad (16B aligned)
            V1 = p_qkv.tile([P, NT, H, HDP], BF16, name="V1")
            nc.gpsimd.memset(V1[:, :, :, HD:HDP], 0.0)
            nc.gpsimd.memset(V1[:, :, :, HD : HD + 1], 1.0)

            es_ctx = ExitStack()
            p_ctx = es_ctx.enter_context(
                tc.tile_pool(name="p_ctx", bufs=1, side="right")
            )
            ctxT = p_ctx.tile([P, KT, N], BF16, name="ctxT")
            es_x1 = ExitStack()
            p_x1 = es_x1.enter_context(
                tc.tile_pool(name="p_x1", bufs=1, side="right")
            )
            x1_sb = p_x1.tile([P, NT, D], FP32, name="x1_sb")
            es_h2 = ExitStack()
            p_h2 = es_h2.enter_context(
                tc.tile_pool(name="p_h2", bufs=1, side="right")
            )
            h2T = p_h2.tile([P, KT, N], BF16, name="h2T")

            # qg-major: all heads of query group 0 first, then qg 1
            units = [
                (2 * hf + s, qg)
                for qg in range(2) for hf in range(KT) for s in range(2)
            ]
            AV_LAG = 2    # units of S/exp emitted ahead of each AV
            TAIL_LAG = 3  # units of S emitted ahead of each normalization tail

            with tc.tile_pool(name="wqk", bufs=2) as wqk_pool, \
                 tc.tile_pool(name="wv", bufs=1) as wv_pool, \
                 tc.tile_pool(name="wp", bufs=1) as wp_pool, \
                 tc.tile_pool(name="etp", bufs=2) as et_pool, \
                 tc.tile_pool(name="ascr", bufs=2) as ascr:

                def qk(oft):
                    wt = wqk_pool.tile([P, KT, P], BF16, tag="wqk", name="wqkt")
                    nc.sync.dma_start(out=wt[:, :, :], in_=wqk_d[oft])
                    for tg in range(2):
                        ps = pt()
                        for kt in range(KT):
                            nc.tensor.matmul(
                                ps[:, :],
                                lhsT=wt[:, kt, :],
                                rhs=h1T[:, kt, ds(tg * 512, 512)],
                                start=(kt == 0),
                                stop=(kt == KT - 1),
                            )
                        if oft < 8:
                            nc.vector.tensor_scalar(
                                out=qT[:, oft, ds(tg * 512, 512)], in0=ps[:, :],
                                scalar1=bqt_sb[:, oft : oft + 1], scalar2=None,
                                op0=ALU.add,
                            )
                        else:
                            hf = oft - 8
                            nc.vector.tensor_scalar(
                                out=kT[:, hf, ds(tg * 512, 512)], in0=ps[:, :],
                                scalar1=bkt_sb[:, hf : hf + 1], scalar2=None,
                                op0=ALU.add,
                            )

                def vblock(og, push_every=0):
                    wvt = wv_pool.tile([P, KT, 512], BF16, tag="wv", name="wvt")
                    nc.sync.dma_start(out=wvt[:, :, :], in_=wv_d[og])
                    for tt in range(NT):
                        ps = pt()
                        for kt in range(KT):
                            nc.tensor.matmul(
                                ps[:, :],
                                lhsT=h1T[:, kt, ts(tt, P)],
                                rhs=wvt[:, kt, :],
                                start=(kt == 0),
                                stop=False,
                            )
                        # fold the v-bias into V so the tail skips its add:
                        # (sum_k et (v+b)) / sum_k et == ctx + b
                        nc.tensor.matmul(
                            ps[:, :],
                            lhsT=ones_bf[0:1, :],
                            rhs=bvt_row[0:1, ds(og * 512, 512)],
                            start=False,
                            stop=True,
                        )
                        nc.vector.tensor_copy(
                            out=V1[:, tt, ds(og * 8, 8), 0:HD],
                            in_=ps[:, :].rearrange("p (h e) -> p h e", e=HD),
                        )
                        if push_every and tt % push_every == push_every - 1:
                            push(1)

                def emit_S(h, qg, et):
                    hf, m0 = h // 2, 64 * (h % 2)
                    for kt in range(KT):
                        ps = pt()
                        nc.tensor.matmul(
                            ps[:, :],
                            lhsT=kT[m0 : m0 + HD, hf, ts(kt, P)],
                            rhs=qT[m0 : m0 + HD, hf, ds(qg * 512, 512)],
                            start=True,
                            stop=True,
                        )
                        nc.scalar.activation(
                            out=et[:, kt, :], in_=ps[:, :], func=AF.Exp,
                            scale=float(HD) ** -0.5,
                        )

                def emit_AV(h, qg, et):
                    psc = pav()
                    for kt in range(KT):
                        nc.tensor.matmul(
                            psc[0:HDP, :],
                            lhsT=V1[:, kt, h, :],
                            rhs=et[:, kt, :],
                            start=(kt == 0),
                            stop=(kt == KT - 1),
                        )
                    # DVE reciprocal is slow but off the critical path via
                    # TAIL_LAG; ACT reciprocal would thrash the activation
                    # table against the softmax Exp (1.3us/swap)
                    rbf = ascr.tile([1, 512], BF16, tag="rbf", name="rbf")
                    with nc.allow_low_precision(reason="softmax denom bf16"):
                        nc.vector.reciprocal(out=rbf[:, :], in_=psc[HD : HD + 1, :])
                    return psc, rbf

                def emit_tail(h, qg, psc, rbf):
                    m0 = 64 * (h % 2)
                    hf = h // 2
                    # broadcast 1/denom across partitions on the idle Pool
                    # engine (frees the PE matmul + a psum bank)
                    rb = ascr.tile([HD, 512], BF16, tag="rb", name="rb")
                    nc.gpsimd.partition_broadcast(
                        out_ap=rb[:, :], in_ap=rbf[0:1, :], channels=HD
                    )
                    if m0 == 0:
                        nc.vector.tensor_tensor(
                            out=ctxT[0:HD, hf, ds(qg * 512, 512)],
                            in0=psc[0:HD, :], in1=rb[:, :], op=ALU.mult,
                        )
                    else:
                        # DVE cannot shift partitions; stage at base 0 then
                        # DMA-shift to partitions 64..127
                        cstg = ascr.tile([HD, 512], BF16, tag="cstg", name="cstg")
                        nc.vector.tensor_tensor(
                            out=cstg[:, :], in0=psc[0:HD, :], in1=rb[:, :],
                            op=ALU.mult,
                        )
                        nc.gpsimd.dma_start(
                            out=ctxT[m0 : m0 + HD, hf, ds(qg * 512, 512)],
                            in_=cstg[:, :],
                        )

                ets, avs = {}, {}
                pipe = {"i": 0}

                def push(n):
                    for _ in range(n):
                        i = pipe["i"]
                        if i >= len(units) + TAIL_LAG:
                            return
                        # AV before S so et slots recycle with bufs=2
                        j = i - AV_LAG
                        if 0 <= j < len(units):
                            avs[j] = emit_AV(*units[j], ets.pop(j))
                        if i < len(units):
                            ets[i] = et_pool.tile(
                                [P, KT, 512], BF16, tag="et", name="et"
                            )
                            emit_S(*units[i], ets[i])
                        k = i - TAIL_LAG
                        if k >= 0 and k in avs:
                            emit_tail(*units[k], *avs.pop(k))
                        pipe["i"] += 1

                wpt = wp_pool.tile([P, 2, KT, 512], BF16, tag="wp", name="wpt")

                def proj_mm(tt):
                    for og in range(2):
                        ps = pt()
                        for kt in range(KT):
                            nc.tensor.matmul(
                                ps[:, :],
                                lhsT=ctxT[:, kt, ts(tt, P)],
                                rhs=wpt[:, og, kt, :],
                                start=(kt == 0),
                                stop=False,
                            )
                        # bias as a K=1 accumulating matmul (frees a DVE pass)
                        nc.tensor.matmul(
                            ps[:, :],
                            lhsT=ones_bf[0:1, :],
                            rhs=bproj_sb[0:1, ds(og * 512, 512)],
                            start=False,
                            stop=True,
                        )
                        nc.vector.tensor_tensor(
                            out=x1_sb[:, tt, ds(og * 512, 512)], in0=ps[:, :],
                            in1=x_sb[:, tt, ds(og * 512, 512)], op=ALU.add,
                        )

                def ln2_tile(tt):
                    xc = scr_ln.tile([P, D], BF16, tag="xc2", name="xc2")
                    ln_stats_xc(x1_sb[:, tt, :], xc[:, :])
                    for ft in range(KT):
                        ln_fm(ft, tt, xc[:, ts(ft, P)], ssT[1][0], ssT[1][1], h2T)

                # schedule: qk pairs hf-interleaved; V blocks + ada2 fill PE
                # while ACT grinds softmax exps; proj for query-group 0
                # overlaps the qg=1 attention units.
                qk(0)
                qk(8)
                vblock(0)
                for hf, np_ in ((1, 1), (2, 2), (3, 1)):
                    qk(hf)
                    qk(8 + hf)
                    push(np_)
                vblock(1, push_every=2)
                for hf in range(4, 8):
                    qk(hf)
                    qk(8 + hf)
                    push(2)
                nc.sync.dma_start(
                    out=wpt[:, :, :, :],
                    in_=wproj_d.rearrange("g p c -> p g c"),
                )
                push(16 + TAIL_LAG - pipe["i"])  # finish qg=0 tails
                for tt in range(4):
                    proj_mm(tt)
                    push(1)
                    ln2_tile(tt)
                    push(1)
                push(len(units) + TAIL_LAG - pipe["i"])
                for tt in range(4, NT):
                    proj_mm(tt)
                    ln2_tile(tt)

            es_qkv.close()  # qT, kT, V1 dead
            es_h1.close()   # h1T, xc1 dead
            es_x.close()    # x dead

            # ---------------- phase G: fc1 + gelu ----------------
            es_f = ExitStack()
            p_f = es_f.enter_context(tc.tile_pool(name="p_f", bufs=1))
            fT = p_f.tile([P, 32, N], BF16, name="fT")
            es_w2 = ExitStack()
            w2_pool = es_w2.enter_context(tc.tile_pool(name="w2", bufs=1))
            w2t0 = w2_pool.tile([P, 32, 512], BF16, tag="w2", name="w2t")
            nc.gpsimd.dma_start(out=w2t0[:, :, :], in_=wfc2_d[0])
            LAG1 = 8
            with tc.tile_pool(name="w1", bufs=LAG1 + 3) as w1_pool, \
                 tc.tile_pool(name="gscr", bufs=6) as gscr:
                # tg0 runs LAG1 ofts ahead of tg1: tg0 only needs the first
                # token half of h2T, so the PE rides out the LN2 tail
                w1ts = {}
                seq = [(k, 0) for k in range(LAG1)]
                for k in range(LAG1, 32):
                    seq += [(k, 0), (k - LAG1, 1)]
                seq += [(k, 1) for k in range(32 - LAG1, 32)]
                for oft, tg in seq:
                    if tg == 0:
                        w1t = w1_pool.tile([P, KT, P], BF16, tag="w1", name="w1t")
                        nc.sync.dma_start(out=w1t[:, :, :], in_=wfc1_d[oft])
                        w1ts[oft] = w1t
                    w1t = w1ts[oft]
                    if True:
                        ps = pt()
                        for kt in range(KT):
                            nc.tensor.matmul(
                                ps[:, :],
                                lhsT=w1t[:, kt, :],
                                rhs=h2T[:, kt, ds(tg * 512, 512)],
                                start=(kt == 0),
                                stop=(kt == KT - 1),
                            )
                        # u = psum + b ; f = (1+approx(u))*u
                        # (the 0.5 of exact gelu is folded into w_fc2)
                        u = gscr.tile([P, 512], FP32, tag="g", name="u")
                        nc.scalar.activation(
                            out=u[:, :], in_=ps[:, :], func=AF.Identity,
                            bias=bfc1t_sb[:, oft : oft + 1],
                        )
                        v = gscr.tile([P, 512], FP32, tag="g", name="v")
                        if GELU_MODE == "erf":
                            # v = erf(u / sqrt(2))
                            nc.scalar.activation(
                                out=v[:, :], in_=ps[:, :], func=AF.Erf,
                                scale=0.7071067811865476,
                                bias=bfc1ts_sb[:, oft : oft + 1],
                            )
                        else:
                            # v = tanh(sqrt(2/pi) * (u + 0.044715 u^3))
                            s = gscr.tile([P, 512], FP32, tag="g", name="s")
                            nc.scalar.activation(
                                out=s[:, :], in_=u[:, :], func=AF.Square
                            )
                            w_ = gscr.tile([P, 512], FP32, tag="g", name="w_")
                            nc.vector.tensor_scalar(
                                out=w_[:, :], in0=s[:, :],
                                scalar1=0.044715 * 0.7978845608028654,
                                scalar2=0.7978845608028654,
                                op0=ALU.mult, op1=ALU.add,
                            )
                            z = gscr.tile([P, 512], FP32, tag="g", name="z")
                            nc.vector.tensor_tensor(
                                out=z[:, :], in0=w_[:, :], in1=u[:, :], op=ALU.mult
                            )
                            nc.scalar.activation(
                                out=v[:, :], in_=z[:, :], func=AF.Tanh
                            )
                        nc.vector.scalar_tensor_tensor(
                            out=fT[:, oft, ds(tg * 512, 512)], in0=v[:, :],
                            scalar=1.0, in1=u[:, :],
                            op0=ALU.add, op1=ALU.mult,
                        )

            es_h2.close()

            # ---------------- phase H: fc2 + residual ----------------
            # chunk1 rides the space h2T freed on the right stack; its DMA
            # hides under chunk0's ~55us of matmuls
            es_w2b = ExitStack()
            w2b_pool = es_w2b.enter_context(
                tc.tile_pool(name="w2b", bufs=1, side="right")
            )
            w2t1 = w2b_pool.tile([P, 32, 512], BF16, name="w2t1")
            nc.gpsimd.dma_start(out=w2t1[:, :, :], in_=wfc2_d[1])
            with tc.tile_pool(name="hscr", bufs=3) as hscr:
                for oc in range(2):
                    w2t = w2t0 if oc == 0 else w2t1
                    for tt in range(NT):
                        ps = pt()
                        for kt in range(32):
                            nc.tensor.matmul(
                                ps[:, :],
                                lhsT=fT[:, kt, ts(tt, P)],
                                rhs=w2t[:, kt, :],
                                start=(kt == 0),
                                stop=False,
                            )
                        nc.tensor.matmul(
                            ps[:, :],
                            lhsT=ones_bf[0:1, :],
                            rhs=bfc2_sb[0:1, ds(oc * 512, 512)],
                            start=False,
                            stop=True,
                        )
                        ot = hscr.tile([P, 512], FP32, tag="ot", name="ot")
                        nc.vector.tensor_tensor(
                            out=ot[:, :], in0=ps[:, :],
                            in1=x1_sb[:, tt, ds(oc * 512, 512)], op=ALU.add,
                        )
                        nc.gpsimd.dma_start(
                            out=out_d[ts(tt, P), ds(oc * 512, 512)], in_=ot[:, :]
                        )

            es_w2b.close()
            es_w2.close()
            es_f.close()
            es_x1.close()
            es_ctx.close()

    nc.compile()
    return nc, names


def _bf(a):
    return np.ascontiguousarray(np.asarray(a, dtype=np.float32)).astype(BF16_NP)


def _f32(a):
    return np.ascontiguousarray(np.asarray(a, dtype=np.float32))


def prep_shared(w):
    """Host-side weight retiling (shared across cores)."""
    wqkv = np.asarray(w["w_qkv"], np.float32)
    bqkv = np.asarray(w["b_qkv"], np.float32)
    bfc1 = np.asarray(w["b_fc1"], np.float32).reshape(32, P).T
    cstf = np.concatenate(
        [
            bqkv[:D].reshape(KT, P).T,           # bqt [P,8]
            bqkv[D : 2 * D].reshape(KT, P).T,    # bkt [P,8]
            bfc1,                                # bfc1t [P,32]
            bfc1 * 0.7071067811865476,           # bfc1ts [P,32]
            np.asarray(w["b_ada1"], np.float32)[:D].reshape(KT, P).T + 1.0,
            np.asarray(w["b_ada1"], np.float32)[D:].reshape(KT, P).T,
            np.asarray(w["b_ada2"], np.float32)[:D].reshape(KT, P).T + 1.0,
            np.asarray(w["b_ada2"], np.float32)[D:].reshape(KT, P).T,
        ],
        axis=1,
    )
    rows = np.concatenate(
        [bqkv[2 * D :], np.asarray(w["b_proj"], np.float32),
         np.asarray(w["b_fc2"], np.float32)]
    ).reshape(1, 3 * D)
    ada1 = np.asarray(w["b_ada1"], np.float32).copy()
    ada1[:D] += 1.0  # pre-add the (1+scale) one
    ada2 = np.asarray(w["b_ada2"], np.float32).copy()
    ada2[:D] += 1.0
    shared = {
        # wqk[oft] = [feat_p(128), kt*128]: column block oft of wq|wk
        "wqk": _bf(
            wqkv[:, : 2 * D].reshape(KT, P, 16, P).transpose(2, 1, 0, 3)
            .reshape(16, P, D)
        ),
        "wv": _bf(
            wqkv[:, 2 * D :].reshape(KT, P, 2, 512).transpose(2, 1, 0, 3)
            .reshape(2, P, 8 * 512)
        ),
        "wproj": _bf(
            np.asarray(w["w_proj"], np.float32)
            .reshape(KT, P, 2, 512).transpose(2, 1, 0, 3).reshape(2, P, 8 * 512)
        ),
        "wada1": _bf(
            np.asarray(w["w_ada1"], np.float32)
            .reshape(KT, P, 4, 512).transpose(2, 1, 0, 3).reshape(4, P, 8 * 512)
        ),
        "wada2": _bf(
            np.asarray(w["w_ada2"], np.float32)
            .reshape(KT, P, 4, 512).transpose(2, 1, 0, 3).reshape(4, P, 8 * 512)
        ),
        "wfc1": _bf(
            np.asarray(w["w_fc1"], np.float32)
            .reshape(KT, P, 32, P).transpose(2, 1, 0, 3).reshape(32, P, D)
        ),
        "wfc2": _bf(
            (np.asarray(w["w_fc2"], np.float32) * 0.5)
            .reshape(32, P, 2, 512).transpose(2, 1, 0, 3).reshape(2, P, 32 * 512)
        ),
        "cstf": _f32(cstf),
        "rows": _bf(rows),
    }
    return shared


def make_in_maps(inputs, names):
    x = np.asarray(inputs["x"], np.float32)
    cond = np.asarray(inputs["condition"], np.float32)
    shared = prep_shared(inputs)
    in_maps = []
    for b in range(B):
        m = {
            names["x"]: _bf(x[b]),
            names["cstb"]: _bf(cond[b].reshape(KT, P).T),
        }
        for k, v in shared.items():
            m[names[k]] = v
        in_maps.append(m)
    return in_maps


_CACHE = {}


def kernel(**inputs) -> np.ndarray:
    if "nc" not in _CACHE:
        _CACHE["nc"], _CACHE["names"] = build()
    nc, names = _CACHE["nc"], _CACHE["names"]
    from concourse.bass_utils import run_bass_kernel_spmd

    in_maps = make_in_maps(inputs, names)
    res = run_bass_kernel_spmd(nc, in_maps, core_ids=list(range(B)))
    out = np.stack([np.asarray(res.results[b][names["out"]]) for b in range(B)])
    return out.astype(np.float32)


if __name__ == "__main__":
    nc, names = build()
    print("built ok:", len(names), "tensors")
